# revision 22
# baseline (speedup 1.0000x reference)
"""Batched zero-phase Butterworth lowpass (filtfilt) on Trainium2.

The per-row map x -> y is linear; the two-sided impulse response g decays
as r^|d| (r ~ 0.82), negligible beyond |d| = 64 at bf16 precision. Each
128-wide output tile y[:, 128k:128k+128] is W_k0^T @ xw_k + W_k1^T @
xw_{k+1}, where xw_m is the 128-sample input window at offset 128m - 64
(transposed) and the W matrices carry the band (exact edge operators for
the first/last 3 tiles, built numerically on host from b, a, zi).

I/O runs in bf16 (tolerance is 2e-2; bf16 end-to-end costs ~3e-3). The
host zero-pads x by 64 columns per side, casts to bf16 and TRANSPOSES
each core's shard to [8320, 512], so the device streams time-major
windows with plain line-rate DMAs and does no transposes at all: W is
the stationary matmul operand (reloaded only ~2x per 8-tile PSUM batch),
x^T windows are the 512-wide moving operand, and y^T tiles go back to
HBM in bf16, transposed and cast to f32 on host. 512 rows per core."""

import sys

for _p in ("/opt/trn_rl_repo",):
    if _p not in sys.path:
        sys.path.insert(0, _p)

import numpy as np
import ml_dtypes

import concourse.bass as bass
import concourse.tile as tile
from concourse import bacc
from concourse import mybir
from concourse.bass_utils import run_bass_kernel_spmd

N = 8192
ROWS = 4096
NCORES = 8
RPC = ROWS // NCORES          # 512 rows per core
F = 128                       # output tile width
NT = N // F                   # 64 output tiles per row
PAD = 64
NP = NT + 1                   # 65 input windows of 128 samples
NPADC = PAD + N + PAD         # 8320 padded columns
S = 3                         # exact special tiles per side
WB = 448                      # exact edge-operator column width
BATCH = 4                     # output tiles per psum/drain/store batch
LOADCH = 8                    # windows per load DMA chunk
PADLEN = 18

_mats_cache = {}
_nc_cache = {}


def _lfilter_batch(b, a, X, Zi):
    z = Zi.copy()
    Y = np.empty_like(X)
    b1, bm, bl = b[0], b[1:-1], b[-1]
    am, al = a[1:-1], a[-1]
    for t in range(X.shape[1]):
        xt = X[:, t]
        y = b1 * xt + z[:, 0]
        Y[:, t] = y
        z[:, :-1] = z[:, 1:] + np.outer(xt, bm) - np.outer(y, am)
        z[:, -1] = bl * xt - al * y
    return Y


def _filtfilt_batch(b, a, zi, X):
    left = 2 * X[:, :1] - X[:, PADLEN:0:-1]
    right = 2 * X[:, -1:] - X[:, -2:-(PADLEN + 2):-1]
    ext = np.concatenate([left, X, right], axis=1)
    y = _lfilter_batch(b, a, ext, np.outer(ext[:, 0], zi))
    y = _lfilter_batch(b, a, y[:, ::-1], np.outer(y[:, -1], zi))[:, ::-1]
    return y[:, PADLEN:-PADLEN]


def _widx(k):
    """Index pair into the stacked weight tensor for output tile k."""
    if k < S:
        return (2 + 2 * k, 3 + 2 * k)
    if k >= NT - S:
        kk = k - (NT - S)
        return (2 + 2 * S + 2 * kk, 3 + 2 * S + 2 * kk)
    return (0, 1)


def _build_mats(b, a, zi):
    """Stacked rhs matrices [2 + 4*S, 128, 128] bf16: interior pair plus
    exact pairs for the first/last S output tiles."""
    key = (b.tobytes(), a.tobytes(), zi.tobytes())
    if key in _mats_cache:
        return _mats_cache[key]
    b64, a64, zi64 = (np.asarray(v, np.float64) for v in (b, a, zi))

    j0 = N // 2
    basis = np.zeros((2 * WB + 1, N))
    for i in range(WB):
        basis[i, i] = 1.0
        basis[WB + i, N - WB + i] = 1.0
    basis[2 * WB, j0] = 1.0
    cols = _filtfilt_batch(b64, a64, zi64, basis)
    g = cols[2 * WB]            # band value g[d] = g[j0 + d]
    Mleft = cols[:WB].T         # [N, WB]  exact M[t, j], j < WB
    Mright = cols[WB:2 * WB].T  # [N, WB]  exact M[t, N - WB + j]

    def gband(d):
        dd = np.clip(j0 + d, 0, N - 1)
        out = g[dd]
        out[np.abs(d) > 200] = 0.0
        return out

    c = np.arange(128)[:, None]
    f = np.arange(128)[None, :]
    mats = [gband(128 * w - 64 + c - f) for w in (0, 1)]

    def special(k):
        R = []
        tt = F * k + f
        for w in (0, 1):
            jj = np.broadcast_to(128 * (k + w) - 64 + c, (128, 128))
            valid = (jj >= 0) & (jj < N)
            jcl = np.clip(jj, 0, N - 1)
            use_left = jcl < WB
            use_right = jcl >= N - WB
            band = gband(jj - np.broadcast_to(tt, jj.shape))
            lw = Mleft[np.broadcast_to(tt, jj.shape), np.where(use_left, jcl, 0)]
            rw = Mright[np.broadcast_to(tt, jj.shape),
                        np.where(use_right, jcl - (N - WB), 0)]
            Rw = np.where(use_left, lw, np.where(use_right, rw, band))
            Rw[~valid] = 0.0
            R.append(Rw)
        return R

    for k in range(S):
        mats += special(k)
    for k in range(NT - S, NT):
        mats += special(k)
    wts = np.stack(mats).astype(np.float32).astype(ml_dtypes.bfloat16)
    # partition-major: [NW, 128 j, 128 t] -> [128 j, NW * 128]
    wts = np.ascontiguousarray(wts.transpose(1, 0, 2).reshape(128, -1))
    _mats_cache[key] = wts
    return wts


def _build_nc():
    if "nc" in _nc_cache:
        return _nc_cache["nc"]
    f32 = mybir.dt.float32
    bf16 = mybir.dt.bfloat16
    NW = 2 + 4 * S
    nc = bacc.Bacc()
    # partition-major: x^T padded as [128 p][65 m][512 r] so every DMA is
    # one long contiguous run per partition (descriptor-gen is the limit)
    xt_in = nc.declare_dram_parameter("xt", [128, NP * RPC], bf16, isOutput=False)
    wts_in = nc.declare_dram_parameter("wts", [128, NW * 128], bf16, isOutput=False)
    yt_out = nc.declare_dram_parameter("yt", [128, NT * RPC], bf16, isOutput=True)

    NCH = 3 + (NP - 9 + LOADCH - 1) // LOADCH  # 3 small + 7 big load chunks
    with tile.TileContext(nc) as tc:
        with (
            tc.tile_pool(name="const", bufs=1) as constp,
            tc.tile_pool(name="xwp", bufs=NCH) as xwp,
            tc.tile_pool(name="ytp", bufs=6) as ytp,
            tc.tile_pool(name="psc", bufs=8, space="PSUM") as psc,
        ):
            wt_all = constp.tile([128, NW * 128], bf16, tag="wt_all")
            nc.scalar.dma_start(wt_all[:, :], wts_in[:, :])
            wtiles = [wt_all[:, i * 128:(i + 1) * 128] for i in range(NW)]

            # all 65 windows fit in SBUF; stream them in upfront.  The
            # first batch's 9 windows go as three small DMAs so compute
            # starts early; the rest as 8-window chunks.
            chunks = [(0, 3), (3, 3), (6, 3)]
            m0 = 9
            while m0 < NP:
                nw = min(LOADCH, NP - m0)
                chunks.append((m0, nw))
                m0 += nw
            win2buf = {}
            for cs, nw in chunks:
                t = xwp.tile([128, nw * RPC], bf16, tag="xw", name="xw")
                nc.sync.dma_start(
                    t[:, :], xt_in[:, RPC * cs:RPC * (cs + nw)]
                )
                for o in range(nw):
                    win2buf[cs + o] = t[:, o * RPC:(o + 1) * RPC]

            def xwin(m):
                return win2buf[m]

            for b0 in range(0, NT, BATCH):
                ks = range(b0, b0 + BATCH)
                pss = {}
                for k in ks:
                    pss[k] = psc.tile([128, RPC], f32, tag="ps", name="ps")
                # first-window pass, then second-window pass, so the
                # shared interior stationary is loaded once per pass
                for w in (0, 1):
                    for k in ks:
                        nc.tensor.matmul(
                            pss[k][:, :], wtiles[_widx(k)[w]], xwin(k + w),
                            start=(w == 0), stop=(w == 1),
                        )
                ybuf = ytp.tile([128, BATCH * RPC], bf16, tag="ybuf")
                for k in ks:
                    o = k - b0
                    dst = ybuf[:, o * RPC:(o + 1) * RPC]
                    if k % 2 == 0:
                        nc.vector.tensor_copy(dst, pss[k][:, :])
                    else:
                        nc.scalar.copy(dst, pss[k][:, :])
                nc.scalar.dma_start(
                    yt_out[:, RPC * b0:RPC * (b0 + BATCH)], ybuf[:, :]
                )
    nc.compile()
    _nc_cache["nc"] = nc
    return nc


def _run(inputs, trace=False, trace_kwargs=None):
    x = np.asarray(inputs["x"], np.float32)
    b = np.asarray(inputs["b"], np.float32)
    a = np.asarray(inputs["a"], np.float32)
    zi = np.asarray(inputs["zi"], np.float32)
    wts = _build_mats(b, a, zi)
    nc = _build_nc()
    in_maps = []
    for i in range(NCORES):
        xt = np.zeros((NPADC, RPC), ml_dtypes.bfloat16)
        xt[PAD:PAD + N, :] = x[i * RPC:(i + 1) * RPC, :].T.astype(
            ml_dtypes.bfloat16
        )
        # partition-major reorder: [65*128, 512] -> [128, 65, 512]
        xh = np.ascontiguousarray(
            xt.reshape(NP, 128, RPC).transpose(1, 0, 2)
        ).reshape(128, NP * RPC)
        in_maps.append({"xt": xh, "wts": wts})
    res = run_bass_kernel_spmd(
        nc, in_maps, list(range(NCORES)), trace=trace,
        **(trace_kwargs or {}),
    )
    y = np.empty((ROWS, N), np.float32)
    for i in range(NCORES):
        yh = res.results[i]["yt"].reshape(128, NT, RPC)
        # [128 p, 64 m, 512 r] -> y[r, 128 m + p]
        y[i * RPC:(i + 1) * RPC, :] = (
            yh.transpose(1, 0, 2).reshape(N, RPC).T.astype(np.float32)
        )
    return y, res


def kernel(**inputs) -> np.ndarray:
    y, _ = _run(inputs, trace=False)
    if not np.isfinite(y).all():
        # one retry on a transient bad execution
        y, _ = _run(inputs, trace=False)
    return y


# revision 24
# speedup vs baseline: 1.0390x; 1.0390x over previous
"""Batched zero-phase Butterworth lowpass (filtfilt) on Trainium2.

The per-row map x -> y is linear; the two-sided impulse response g decays
as r^|d| (r ~ 0.82), negligible beyond |d| = 64 at bf16 precision. Each
128-wide output tile y[:, 128k:128k+128] is W_k0^T @ xw_k + W_k1^T @
xw_{k+1}, where xw_m is the 128-sample input window at offset 128m - 64
(transposed) and the W matrices carry the band (exact edge operators for
the first/last 3 tiles, built numerically on host from b, a, zi).

I/O runs in bf16 (tolerance is 2e-2; bf16 end-to-end costs ~3e-3). The
host zero-pads x by 64 columns per side, casts to bf16 and TRANSPOSES
each core's shard to [8320, 512], so the device streams time-major
windows with plain line-rate DMAs and does no transposes at all: W is
the stationary matmul operand (reloaded only ~2x per 8-tile PSUM batch),
x^T windows are the 512-wide moving operand, and y^T tiles go back to
HBM in bf16, transposed and cast to f32 on host. 512 rows per core."""

import sys

for _p in ("/opt/trn_rl_repo",):
    if _p not in sys.path:
        sys.path.insert(0, _p)

import numpy as np
import ml_dtypes

import concourse.bass as bass
import concourse.tile as tile
from concourse import bacc
from concourse import mybir
from concourse.bass_utils import run_bass_kernel_spmd

N = 8192
ROWS = 4096
NCORES = 8
RPC = ROWS // NCORES          # 512 rows per core
F = 128                       # output tile width
NT = N // F                   # 64 output tiles per row
PAD = 64
NP = NT + 1                   # 65 input windows of 128 samples
NPADC = PAD + N + PAD         # 8320 padded columns
S = 3                         # exact special tiles per side
WB = 448                      # exact edge-operator column width
BATCH = 4                     # output tiles per psum/drain/store batch
LOADCH = 8                    # windows per load DMA chunk
PADLEN = 18

_mats_cache = {}
_nc_cache = {}


def _lfilter_batch(b, a, X, Zi):
    z = Zi.copy()
    Y = np.empty_like(X)
    b1, bm, bl = b[0], b[1:-1], b[-1]
    am, al = a[1:-1], a[-1]
    for t in range(X.shape[1]):
        xt = X[:, t]
        y = b1 * xt + z[:, 0]
        Y[:, t] = y
        z[:, :-1] = z[:, 1:] + np.outer(xt, bm) - np.outer(y, am)
        z[:, -1] = bl * xt - al * y
    return Y


def _filtfilt_batch(b, a, zi, X):
    left = 2 * X[:, :1] - X[:, PADLEN:0:-1]
    right = 2 * X[:, -1:] - X[:, -2:-(PADLEN + 2):-1]
    ext = np.concatenate([left, X, right], axis=1)
    y = _lfilter_batch(b, a, ext, np.outer(ext[:, 0], zi))
    y = _lfilter_batch(b, a, y[:, ::-1], np.outer(y[:, -1], zi))[:, ::-1]
    return y[:, PADLEN:-PADLEN]


def _widx(k):
    """Index pair into the stacked weight tensor for output tile k."""
    if k < S:
        return (2 + 2 * k, 3 + 2 * k)
    if k >= NT - S:
        kk = k - (NT - S)
        return (2 + 2 * S + 2 * kk, 3 + 2 * S + 2 * kk)
    return (0, 1)


def _build_mats(b, a, zi):
    """Stacked rhs matrices [2 + 4*S, 128, 128] bf16: interior pair plus
    exact pairs for the first/last S output tiles."""
    key = (b.tobytes(), a.tobytes(), zi.tobytes())
    if key in _mats_cache:
        return _mats_cache[key]
    b64, a64, zi64 = (np.asarray(v, np.float64) for v in (b, a, zi))

    j0 = N // 2
    basis = np.zeros((2 * WB + 1, N))
    for i in range(WB):
        basis[i, i] = 1.0
        basis[WB + i, N - WB + i] = 1.0
    basis[2 * WB, j0] = 1.0
    cols = _filtfilt_batch(b64, a64, zi64, basis)
    g = cols[2 * WB]            # band value g[d] = g[j0 + d]
    Mleft = cols[:WB].T         # [N, WB]  exact M[t, j], j < WB
    Mright = cols[WB:2 * WB].T  # [N, WB]  exact M[t, N - WB + j]

    def gband(d):
        dd = np.clip(j0 + d, 0, N - 1)
        out = g[dd]
        out[np.abs(d) > 200] = 0.0
        return out

    c = np.arange(128)[:, None]
    f = np.arange(128)[None, :]
    mats = [gband(128 * w - 64 + c - f) for w in (0, 1)]

    def special(k):
        R = []
        tt = F * k + f
        for w in (0, 1):
            jj = np.broadcast_to(128 * (k + w) - 64 + c, (128, 128))
            valid = (jj >= 0) & (jj < N)
            jcl = np.clip(jj, 0, N - 1)
            use_left = jcl < WB
            use_right = jcl >= N - WB
            band = gband(jj - np.broadcast_to(tt, jj.shape))
            lw = Mleft[np.broadcast_to(tt, jj.shape), np.where(use_left, jcl, 0)]
            rw = Mright[np.broadcast_to(tt, jj.shape),
                        np.where(use_right, jcl - (N - WB), 0)]
            Rw = np.where(use_left, lw, np.where(use_right, rw, band))
            Rw[~valid] = 0.0
            R.append(Rw)
        return R

    for k in range(S):
        mats += special(k)
    for k in range(NT - S, NT):
        mats += special(k)
    wts = np.stack(mats).astype(np.float32).astype(ml_dtypes.bfloat16)
    # partition-major: [NW, 128 j, 128 t] -> [128 j, NW * 128]
    wts = np.ascontiguousarray(wts.transpose(1, 0, 2).reshape(128, -1))
    _mats_cache[key] = wts
    return wts


def _build_nc():
    if "nc" in _nc_cache:
        return _nc_cache["nc"]
    f32 = mybir.dt.float32
    bf16 = mybir.dt.bfloat16
    NW = 2 + 4 * S
    nc = bacc.Bacc()
    # partition-major: x^T padded as [128 p][65 m][512 r] so every DMA is
    # one long contiguous run per partition (descriptor-gen is the limit)
    xt_in = nc.declare_dram_parameter("xt", [128, NP * RPC], bf16, isOutput=False)
    wts_in = nc.declare_dram_parameter("wts", [128, NW * 128], bf16, isOutput=False)
    yt_out = nc.declare_dram_parameter("yt", [128, NT * RPC], bf16, isOutput=True)

    NCH = 3 + (NP - 9 + LOADCH - 1) // LOADCH  # 3 small + 7 big load chunks
    with tile.TileContext(nc) as tc:
        with (
            tc.tile_pool(name="const", bufs=1) as constp,
            tc.tile_pool(name="xwp", bufs=6) as xwp,
            tc.tile_pool(name="ytp", bufs=6) as ytp,
            tc.tile_pool(name="psc", bufs=8, space="PSUM") as psc,
        ):
            wt_all = constp.tile([128, NW * 128], bf16, tag="wt_all")
            nc.scalar.dma_start(wt_all[:, :], wts_in[:, :])
            wtiles = [wt_all[:, i * 128:(i + 1) * 128] for i in range(NW)]

            # all 65 windows fit in SBUF; stream them in upfront.  The
            # first batch's 9 windows go as three small DMAs so compute
            # starts early; the rest as 8-window chunks.
            chunks = [(0, 3), (3, 3), (6, 3)]
            m0 = 9
            while m0 < NP:
                nw = min(LOADCH, NP - m0)
                chunks.append((m0, nw))
                m0 += nw
            win2buf = {}
            for cs, nw in chunks:
                t = xwp.tile([128, nw * RPC], bf16, tag="xw", name="xw")
                nc.sync.dma_start(
                    t[:, :], xt_in[:, RPC * cs:RPC * (cs + nw)]
                )
                for o in range(nw):
                    win2buf[cs + o] = t[:, o * RPC:(o + 1) * RPC]

            def xwin(m):
                return win2buf[m]

            for b0 in range(0, NT, BATCH):
                ks = range(b0, b0 + BATCH)
                pss = {}
                for k in ks:
                    pss[k] = psc.tile([128, RPC], f32, tag="ps", name="ps")
                # first-window pass, then second-window pass, so the
                # shared interior stationary is loaded once per pass
                for w in (0, 1):
                    for k in ks:
                        nc.tensor.matmul(
                            pss[k][:, :], wtiles[_widx(k)[w]], xwin(k + w),
                            start=(w == 0), stop=(w == 1),
                        )
                ybuf = ytp.tile([128, BATCH * RPC], bf16, tag="ybuf")
                H = RPC // 2
                for k in ks:
                    o = k - b0
                    dst = ybuf[:, o * RPC:(o + 1) * RPC]
                    # split each bank drain across both engines so the
                    # psum bank frees in half the time
                    nc.vector.tensor_copy(dst[:, :H], pss[k][:, :H])
                    nc.scalar.copy(dst[:, H:], pss[k][:, H:])
                nc.scalar.dma_start(
                    yt_out[:, RPC * b0:RPC * (b0 + BATCH)], ybuf[:, :]
                )
    nc.compile()
    _nc_cache["nc"] = nc
    return nc


def _run(inputs, trace=False, trace_kwargs=None):
    x = np.asarray(inputs["x"], np.float32)
    b = np.asarray(inputs["b"], np.float32)
    a = np.asarray(inputs["a"], np.float32)
    zi = np.asarray(inputs["zi"], np.float32)
    wts = _build_mats(b, a, zi)
    nc = _build_nc()
    in_maps = []
    for i in range(NCORES):
        xt = np.zeros((NPADC, RPC), ml_dtypes.bfloat16)
        xt[PAD:PAD + N, :] = x[i * RPC:(i + 1) * RPC, :].T.astype(
            ml_dtypes.bfloat16
        )
        # partition-major reorder: [65*128, 512] -> [128, 65, 512]
        xh = np.ascontiguousarray(
            xt.reshape(NP, 128, RPC).transpose(1, 0, 2)
        ).reshape(128, NP * RPC)
        in_maps.append({"xt": xh, "wts": wts})
    res = run_bass_kernel_spmd(
        nc, in_maps, list(range(NCORES)), trace=trace,
        **(trace_kwargs or {}),
    )
    y = np.empty((ROWS, N), np.float32)
    for i in range(NCORES):
        yh = res.results[i]["yt"].reshape(128, NT, RPC)
        # [128 p, 64 m, 512 r] -> y[r, 128 m + p]
        y[i * RPC:(i + 1) * RPC, :] = (
            yh.transpose(1, 0, 2).reshape(N, RPC).T.astype(np.float32)
        )
    return y, res


def kernel(**inputs) -> np.ndarray:
    y, _ = _run(inputs, trace=False)
    if not np.isfinite(y).all():
        # one retry on a transient bad execution
        y, _ = _run(inputs, trace=False)
    return y


# revision 26
# speedup vs baseline: 1.0458x; 1.0065x over previous
"""Batched zero-phase Butterworth lowpass (filtfilt) on Trainium2.

The per-row map x -> y is linear; the two-sided impulse response g decays
as r^|d| (r ~ 0.82), negligible beyond |d| = 64 at bf16 precision. Each
128-wide output tile y[:, 128k:128k+128] is W_k0^T @ xw_k + W_k1^T @
xw_{k+1}, where xw_m is the 128-sample input window at offset 128m - 64
(transposed) and the W matrices carry the band (exact edge operators for
the first/last 3 tiles, built numerically on host from b, a, zi).

I/O runs in bf16 (tolerance is 2e-2; bf16 end-to-end costs ~3e-3). The
host zero-pads x by 64 columns per side, casts to bf16 and TRANSPOSES
each core's shard to [8320, 512], so the device streams time-major
windows with plain line-rate DMAs and does no transposes at all: W is
the stationary matmul operand (reloaded only ~2x per 8-tile PSUM batch),
x^T windows are the 512-wide moving operand, and y^T tiles go back to
HBM in bf16, transposed and cast to f32 on host. 512 rows per core."""

import sys

for _p in ("/opt/trn_rl_repo",):
    if _p not in sys.path:
        sys.path.insert(0, _p)

import numpy as np
import ml_dtypes

import concourse.bass as bass
import concourse.tile as tile
from concourse import bacc
from concourse import mybir
from concourse.bass_utils import run_bass_kernel_spmd

N = 8192
ROWS = 4096
NCORES = 8
RPC = ROWS // NCORES          # 512 rows per core
F = 128                       # output tile width
NT = N // F                   # 64 output tiles per row
PAD = 64
NP = NT + 1                   # 65 input windows of 128 samples
NPADC = PAD + N + PAD         # 8320 padded columns
S = 3                         # exact special tiles per side
WB = 448                      # exact edge-operator column width
BATCH = 4                     # output tiles per psum/drain/store batch
LOADCH = 8                    # windows per load DMA chunk
PADLEN = 18

_mats_cache = {}
_nc_cache = {}


def _lfilter_batch(b, a, X, Zi):
    z = Zi.copy()
    Y = np.empty_like(X)
    b1, bm, bl = b[0], b[1:-1], b[-1]
    am, al = a[1:-1], a[-1]
    for t in range(X.shape[1]):
        xt = X[:, t]
        y = b1 * xt + z[:, 0]
        Y[:, t] = y
        z[:, :-1] = z[:, 1:] + np.outer(xt, bm) - np.outer(y, am)
        z[:, -1] = bl * xt - al * y
    return Y


def _filtfilt_batch(b, a, zi, X):
    left = 2 * X[:, :1] - X[:, PADLEN:0:-1]
    right = 2 * X[:, -1:] - X[:, -2:-(PADLEN + 2):-1]
    ext = np.concatenate([left, X, right], axis=1)
    y = _lfilter_batch(b, a, ext, np.outer(ext[:, 0], zi))
    y = _lfilter_batch(b, a, y[:, ::-1], np.outer(y[:, -1], zi))[:, ::-1]
    return y[:, PADLEN:-PADLEN]


def _widx(k):
    """Index pair into the stacked weight tensor for output tile k."""
    if k < S:
        return (2 + 2 * k, 3 + 2 * k)
    if k >= NT - S:
        kk = k - (NT - S)
        return (2 + 2 * S + 2 * kk, 3 + 2 * S + 2 * kk)
    return (0, 1)


def _build_mats(b, a, zi):
    """Stacked rhs matrices [2 + 4*S, 128, 128] bf16: interior pair plus
    exact pairs for the first/last S output tiles."""
    key = (b.tobytes(), a.tobytes(), zi.tobytes())
    if key in _mats_cache:
        return _mats_cache[key]
    b64, a64, zi64 = (np.asarray(v, np.float64) for v in (b, a, zi))

    j0 = N // 2
    basis = np.zeros((2 * WB + 1, N))
    for i in range(WB):
        basis[i, i] = 1.0
        basis[WB + i, N - WB + i] = 1.0
    basis[2 * WB, j0] = 1.0
    cols = _filtfilt_batch(b64, a64, zi64, basis)
    g = cols[2 * WB]            # band value g[d] = g[j0 + d]
    Mleft = cols[:WB].T         # [N, WB]  exact M[t, j], j < WB
    Mright = cols[WB:2 * WB].T  # [N, WB]  exact M[t, N - WB + j]

    def gband(d):
        dd = np.clip(j0 + d, 0, N - 1)
        out = g[dd]
        out[np.abs(d) > 200] = 0.0
        return out

    c = np.arange(128)[:, None]
    f = np.arange(128)[None, :]
    mats = [gband(128 * w - 64 + c - f) for w in (0, 1)]

    def special(k):
        R = []
        tt = F * k + f
        for w in (0, 1):
            jj = np.broadcast_to(128 * (k + w) - 64 + c, (128, 128))
            valid = (jj >= 0) & (jj < N)
            jcl = np.clip(jj, 0, N - 1)
            use_left = jcl < WB
            use_right = jcl >= N - WB
            band = gband(jj - np.broadcast_to(tt, jj.shape))
            lw = Mleft[np.broadcast_to(tt, jj.shape), np.where(use_left, jcl, 0)]
            rw = Mright[np.broadcast_to(tt, jj.shape),
                        np.where(use_right, jcl - (N - WB), 0)]
            Rw = np.where(use_left, lw, np.where(use_right, rw, band))
            Rw[~valid] = 0.0
            R.append(Rw)
        return R

    for k in range(S):
        mats += special(k)
    for k in range(NT - S, NT):
        mats += special(k)
    wts = np.stack(mats).astype(np.float32).astype(ml_dtypes.bfloat16)
    # partition-major: [NW, 128 j, 128 t] -> [128 j, NW * 128]
    wts = np.ascontiguousarray(wts.transpose(1, 0, 2).reshape(128, -1))
    _mats_cache[key] = wts
    return wts


def _build_nc():
    if "nc" in _nc_cache:
        return _nc_cache["nc"]
    f32 = mybir.dt.float32
    bf16 = mybir.dt.bfloat16
    NW = 2 + 4 * S
    nc = bacc.Bacc()
    # partition-major: x^T padded as [128 p][65 m][512 r] so every DMA is
    # one long contiguous run per partition (descriptor-gen is the limit)
    xt_in = nc.declare_dram_parameter("xt", [128, NP * RPC], bf16, isOutput=False)
    wts_in = nc.declare_dram_parameter("wts", [128, NW * 128], bf16, isOutput=False)
    yt_out = nc.declare_dram_parameter("yt", [128, NT * RPC], bf16, isOutput=True)

    NCH = 3 + (NP - 9 + LOADCH - 1) // LOADCH  # 3 small + 7 big load chunks
    with tile.TileContext(nc) as tc:
        with (
            tc.tile_pool(name="const", bufs=1) as constp,
            tc.tile_pool(name="xwp", bufs=6) as xwp,
            tc.tile_pool(name="ytp", bufs=10) as ytp,
            tc.tile_pool(name="psc", bufs=8, space="PSUM") as psc,
        ):
            wt_all = constp.tile([128, NW * 128], bf16, tag="wt_all")
            nc.sync.dma_start(wt_all[:, :], wts_in[:, :])
            wtiles = [wt_all[:, i * 128:(i + 1) * 128] for i in range(NW)]

            # dummy matmuls with no data dependencies: ramp the PE clock
            # gate during the pipeline-fill window
            warm = constp.tile([128, RPC], bf16, tag="warm")
            nc.gpsimd.memset(warm[:, :], 0.0)
            ps_warm = psc.tile([128, RPC], f32, tag="ps", name="ps")
            for _ in range(16):
                nc.tensor.matmul(
                    ps_warm[:, :], warm[:, 0:128], warm[:, :],
                    start=True, stop=True,
                )

            # all 65 windows fit in SBUF; stream them in upfront.  The
            # first batch's 9 windows go as three small DMAs so compute
            # starts early; the rest as 8-window chunks.
            chunks = [(0, 3), (3, 3), (6, 3)]
            m0 = 9
            while m0 < NP:
                nw = min(LOADCH, NP - m0)
                chunks.append((m0, nw))
                m0 += nw
            win2buf = {}
            for cs, nw in chunks:
                t = xwp.tile([128, nw * RPC], bf16, tag="xw", name="xw")
                nc.sync.dma_start(
                    t[:, :], xt_in[:, RPC * cs:RPC * (cs + nw)]
                )
                for o in range(nw):
                    win2buf[cs + o] = t[:, o * RPC:(o + 1) * RPC]

            def xwin(m):
                return win2buf[m]

            for b0 in range(0, NT, BATCH):
                ks = range(b0, b0 + BATCH)
                pss = {}
                for k in ks:
                    pss[k] = psc.tile([128, RPC], f32, tag="ps", name="ps")
                # first-window pass, then second-window pass, so the
                # shared interior stationary is loaded once per pass
                for w in (0, 1):
                    for k in ks:
                        nc.tensor.matmul(
                            pss[k][:, :], wtiles[_widx(k)[w]], xwin(k + w),
                            start=(w == 0), stop=(w == 1),
                        )
                ybuf = ytp.tile([128, BATCH * RPC], bf16, tag="ybuf")
                H = RPC // 2
                for k in ks:
                    o = k - b0
                    dst = ybuf[:, o * RPC:(o + 1) * RPC]
                    # split each bank drain across both engines so the
                    # psum bank frees in half the time
                    nc.vector.tensor_copy(dst[:, :H], pss[k][:, :H])
                    nc.scalar.copy(dst[:, H:], pss[k][:, H:])
                nc.scalar.dma_start(
                    yt_out[:, RPC * b0:RPC * (b0 + BATCH)], ybuf[:, :]
                )
    nc.compile()
    _nc_cache["nc"] = nc
    return nc


def _run(inputs, trace=False, trace_kwargs=None):
    x = np.asarray(inputs["x"], np.float32)
    b = np.asarray(inputs["b"], np.float32)
    a = np.asarray(inputs["a"], np.float32)
    zi = np.asarray(inputs["zi"], np.float32)
    wts = _build_mats(b, a, zi)
    nc = _build_nc()
    in_maps = []
    for i in range(NCORES):
        xt = np.zeros((NPADC, RPC), ml_dtypes.bfloat16)
        xt[PAD:PAD + N, :] = x[i * RPC:(i + 1) * RPC, :].T.astype(
            ml_dtypes.bfloat16
        )
        # partition-major reorder: [65*128, 512] -> [128, 65, 512]
        xh = np.ascontiguousarray(
            xt.reshape(NP, 128, RPC).transpose(1, 0, 2)
        ).reshape(128, NP * RPC)
        in_maps.append({"xt": xh, "wts": wts})
    res = run_bass_kernel_spmd(
        nc, in_maps, list(range(NCORES)), trace=trace,
        **(trace_kwargs or {}),
    )
    y = np.empty((ROWS, N), np.float32)
    for i in range(NCORES):
        yh = res.results[i]["yt"].reshape(128, NT, RPC)
        # [128 p, 64 m, 512 r] -> y[r, 128 m + p]
        y[i * RPC:(i + 1) * RPC, :] = (
            yh.transpose(1, 0, 2).reshape(N, RPC).T.astype(np.float32)
        )
    return y, res


def kernel(**inputs) -> np.ndarray:
    y, _ = _run(inputs, trace=False)
    if not np.isfinite(y).all():
        # one retry on a transient bad execution
        y, _ = _run(inputs, trace=False)
    return y


# revision 29
# speedup vs baseline: 1.1900x; 1.1379x over previous
"""Batched zero-phase Butterworth lowpass (filtfilt) on Trainium2.

The per-row map x -> y is linear; the two-sided impulse response g decays
as r^|d| (r ~ 0.82), negligible beyond |d| = 64 at bf16 precision. Each
128-wide output tile y[:, 128k:128k+128] is W_k0^T @ xw_k + W_k1^T @
xw_{k+1}, where xw_m is the 128-sample input window at offset 128m - 64
(transposed) and the W matrices carry the band (exact edge operators for
the first/last 3 tiles, built numerically on host from b, a, zi).

I/O runs in bf16 (tolerance is 2e-2; bf16 end-to-end costs ~3e-3). The
host zero-pads x by 64 columns per side, casts to bf16 and TRANSPOSES
each core's shard to [8320, 512], so the device streams time-major
windows with plain line-rate DMAs and does no transposes at all: W is
the stationary matmul operand (reloaded only ~2x per 8-tile PSUM batch),
x^T windows are the 512-wide moving operand, and y^T tiles go back to
HBM in bf16, transposed and cast to f32 on host. 512 rows per core."""

import sys

for _p in ("/opt/trn_rl_repo",):
    if _p not in sys.path:
        sys.path.insert(0, _p)

import numpy as np
import ml_dtypes

import concourse.bass as bass
import concourse.tile as tile
from concourse import bacc
from concourse import mybir
from concourse.bass_utils import run_bass_kernel_spmd

N = 8192
ROWS = 4096
NCORES = 8
RPC = ROWS // NCORES          # 512 rows per core
F = 128                       # output tile width
NT = N // F                   # 64 output tiles per row
PAD = 64
NP = NT + 1                   # 65 input windows of 128 samples
NPADC = PAD + N + PAD         # 8320 padded columns
S = 3                         # exact special tiles per side
WB = 448                      # exact edge-operator column width
BATCH = 4                     # output tiles per psum/drain/store batch
LOADCH = 8                    # windows per load DMA chunk
PADLEN = 18

_mats_cache = {}
_nc_cache = {}


def _lfilter_batch(b, a, X, Zi):
    z = Zi.copy()
    Y = np.empty_like(X)
    b1, bm, bl = b[0], b[1:-1], b[-1]
    am, al = a[1:-1], a[-1]
    for t in range(X.shape[1]):
        xt = X[:, t]
        y = b1 * xt + z[:, 0]
        Y[:, t] = y
        z[:, :-1] = z[:, 1:] + np.outer(xt, bm) - np.outer(y, am)
        z[:, -1] = bl * xt - al * y
    return Y


def _filtfilt_batch(b, a, zi, X):
    left = 2 * X[:, :1] - X[:, PADLEN:0:-1]
    right = 2 * X[:, -1:] - X[:, -2:-(PADLEN + 2):-1]
    ext = np.concatenate([left, X, right], axis=1)
    y = _lfilter_batch(b, a, ext, np.outer(ext[:, 0], zi))
    y = _lfilter_batch(b, a, y[:, ::-1], np.outer(y[:, -1], zi))[:, ::-1]
    return y[:, PADLEN:-PADLEN]


def _widx(k):
    """Index pair into the stacked weight tensor for output tile k."""
    if k < S:
        return (2 + 2 * k, 3 + 2 * k)
    if k >= NT - S:
        kk = k - (NT - S)
        return (2 + 2 * S + 2 * kk, 3 + 2 * S + 2 * kk)
    return (0, 1)


def _build_mats(b, a, zi):
    """Stacked rhs matrices [2 + 4*S, 128, 128] bf16: interior pair plus
    exact pairs for the first/last S output tiles."""
    key = (b.tobytes(), a.tobytes(), zi.tobytes())
    if key in _mats_cache:
        return _mats_cache[key]
    b64, a64, zi64 = (np.asarray(v, np.float64) for v in (b, a, zi))

    j0 = N // 2
    basis = np.zeros((2 * WB + 1, N))
    for i in range(WB):
        basis[i, i] = 1.0
        basis[WB + i, N - WB + i] = 1.0
    basis[2 * WB, j0] = 1.0
    cols = _filtfilt_batch(b64, a64, zi64, basis)
    g = cols[2 * WB]            # band value g[d] = g[j0 + d]
    Mleft = cols[:WB].T         # [N, WB]  exact M[t, j], j < WB
    Mright = cols[WB:2 * WB].T  # [N, WB]  exact M[t, N - WB + j]

    def gband(d):
        dd = np.clip(j0 + d, 0, N - 1)
        out = g[dd]
        out[np.abs(d) > 200] = 0.0
        return out

    c = np.arange(128)[:, None]
    f = np.arange(128)[None, :]
    mats = [gband(128 * w - 64 + c - f) for w in (0, 1)]

    def special(k):
        R = []
        tt = F * k + f
        for w in (0, 1):
            jj = np.broadcast_to(128 * (k + w) - 64 + c, (128, 128))
            valid = (jj >= 0) & (jj < N)
            jcl = np.clip(jj, 0, N - 1)
            use_left = jcl < WB
            use_right = jcl >= N - WB
            band = gband(jj - np.broadcast_to(tt, jj.shape))
            lw = Mleft[np.broadcast_to(tt, jj.shape), np.where(use_left, jcl, 0)]
            rw = Mright[np.broadcast_to(tt, jj.shape),
                        np.where(use_right, jcl - (N - WB), 0)]
            Rw = np.where(use_left, lw, np.where(use_right, rw, band))
            Rw[~valid] = 0.0
            R.append(Rw)
        return R

    for k in range(S):
        mats += special(k)
    for k in range(NT - S, NT):
        mats += special(k)
    wts = np.stack(mats).astype(np.float32).astype(ml_dtypes.bfloat16)
    # partition-major: [NW, 128 j, 128 t] -> [128 j, NW * 128]
    wts = np.ascontiguousarray(wts.transpose(1, 0, 2).reshape(128, -1))
    _mats_cache[key] = wts
    return wts


def _build_nc():
    if "nc" in _nc_cache:
        return _nc_cache["nc"]
    f32 = mybir.dt.float32
    bf16 = mybir.dt.bfloat16
    NW = 2 + 4 * S
    nc = bacc.Bacc()
    # partition-major: x^T padded as [128 p][65 m][512 r] so every DMA is
    # one long contiguous run per partition (descriptor-gen is the limit)
    xt_in = nc.declare_dram_parameter("xt", [128, NP * RPC], bf16, isOutput=False)
    wts_in = nc.declare_dram_parameter("wts", [128, NW * 128], bf16, isOutput=False)
    yt_out = nc.declare_dram_parameter("yt", [128, NT * RPC], bf16, isOutput=True)

    NCH = 3 + (NP - 9 + LOADCH - 1) // LOADCH  # 3 small + 7 big load chunks
    with tile.TileContext(nc) as tc:
        with (
            tc.tile_pool(name="const", bufs=1) as constp,
            tc.tile_pool(name="xwp", bufs=6) as xwp,
            tc.tile_pool(name="ytp", bufs=12) as ytp,
            tc.tile_pool(name="psc", bufs=8, space="PSUM") as psc,
        ):
            wt_all = constp.tile([128, NW * 128], bf16, tag="wt_all")
            nc.sync.dma_start(wt_all[:, :], wts_in[:, :])
            wtiles = [wt_all[:, i * 128:(i + 1) * 128] for i in range(NW)]

            # dummy matmuls: ramp the PE clock gate during the
            # pipeline-fill window (results are never used)
            warm = constp.tile([128, RPC], bf16, tag="warm")
            nc.vector.memset(warm[:, :], 0.0)
            ps_warm = psc.tile([128, RPC], f32, tag="ps", name="ps")
            for _ in range(8):
                nc.tensor.matmul(
                    ps_warm[:, :], warm[:, 0:128], warm[:, :],
                    start=True, stop=True,
                )

            # all 65 windows fit in SBUF; stream them in upfront.  The
            # first batch's 9 windows go as three small DMAs so compute
            # starts early; the rest as 8-window chunks.
            chunks = [(0, 3), (3, 3), (6, 3)]
            m0 = 9
            while m0 < NP:
                nw = min(LOADCH, NP - m0)
                chunks.append((m0, nw))
                m0 += nw
            win2buf = {}
            for cs, nw in chunks:
                t = xwp.tile([128, nw * RPC], bf16, tag="xw", name="xw")
                nc.sync.dma_start(
                    t[:, :], xt_in[:, RPC * cs:RPC * (cs + nw)]
                )
                for o in range(nw):
                    win2buf[cs + o] = t[:, o * RPC:(o + 1) * RPC]

            def xwin(m):
                return win2buf[m]

            for b0 in range(0, NT, BATCH):
                ks = range(b0, b0 + BATCH)
                pss = {}
                for k in ks:
                    pss[k] = psc.tile([128, RPC], f32, tag="ps", name="ps")
                # first-window pass, then second-window pass, so the
                # shared interior stationary is loaded once per pass
                for w in (0, 1):
                    for k in ks:
                        nc.tensor.matmul(
                            pss[k][:, :], wtiles[_widx(k)[w]], xwin(k + w),
                            start=(w == 0), stop=(w == 1),
                        )
                ybuf = ytp.tile([128, BATCH * RPC], bf16, tag="ybuf")
                H = RPC // 2
                for k in ks:
                    o = k - b0
                    dst = ybuf[:, o * RPC:(o + 1) * RPC]
                    # split each bank drain across both engines so the
                    # psum bank frees in half the time
                    nc.vector.tensor_copy(dst[:, :H], pss[k][:, :H])
                    nc.scalar.copy(dst[:, H:], pss[k][:, H:])
                nc.scalar.dma_start(
                    yt_out[:, RPC * b0:RPC * (b0 + BATCH)], ybuf[:, :]
                )
    nc.compile()
    _nc_cache["nc"] = nc
    return nc


def _run(inputs, trace=False, trace_kwargs=None):
    x = np.asarray(inputs["x"], np.float32)
    b = np.asarray(inputs["b"], np.float32)
    a = np.asarray(inputs["a"], np.float32)
    zi = np.asarray(inputs["zi"], np.float32)
    wts = _build_mats(b, a, zi)
    nc = _build_nc()
    in_maps = []
    for i in range(NCORES):
        xt = np.zeros((NPADC, RPC), ml_dtypes.bfloat16)
        xt[PAD:PAD + N, :] = x[i * RPC:(i + 1) * RPC, :].T.astype(
            ml_dtypes.bfloat16
        )
        # partition-major reorder: [65*128, 512] -> [128, 65, 512]
        xh = np.ascontiguousarray(
            xt.reshape(NP, 128, RPC).transpose(1, 0, 2)
        ).reshape(128, NP * RPC)
        in_maps.append({"xt": xh, "wts": wts})
    res = run_bass_kernel_spmd(
        nc, in_maps, list(range(NCORES)), trace=trace,
        **(trace_kwargs or {}),
    )
    y = np.empty((ROWS, N), np.float32)
    for i in range(NCORES):
        yh = res.results[i]["yt"].reshape(128, NT, RPC)
        # [128 p, 64 m, 512 r] -> y[r, 128 m + p]
        y[i * RPC:(i + 1) * RPC, :] = (
            yh.transpose(1, 0, 2).reshape(N, RPC).T.astype(np.float32)
        )
    return y, res


def kernel(**inputs) -> np.ndarray:
    y, _ = _run(inputs, trace=False)
    if not np.isfinite(y).all():
        # one retry on a transient bad execution
        y, _ = _run(inputs, trace=False)
    return y
